# revision 6
# baseline (speedup 1.0000x reference)
"""Trainium2 Bass kernel for MultiHeadPositionBiasBasedForMLM.

Strategy (8 NeuronCores): data-parallel over batch. Core c computes the
full pipeline for batch c: position-bias attention -> dense+gelu+LN ->
decoder logits over the full vocab. The decoder weight (with LayerNorm
gamma folded in, transposed, bf16) is replicated to all cores.

Attention is computed in transposed layout E[j, i] per head so that every
contraction (scores, softmax denominator, alpha @ v) reduces over the
partition dim on the PE. Softmax skips max-subtraction (scores are O(6))
and defers the 1/denom scale until after the alpha @ v matmul. The padding
mask is applied by pre-zeroing masked v rows on the host and using a
mask-weighted ones matrix for the denominator matmul.
"""

import numpy as np
import ml_dtypes

B, H, Q, S = 8, 12, 512, 512
D, HD, V = 768, 64, 768 // 12
HD = D // H
V = 30522
VP = 30720  # vocab padded to 60 tiles of 512
NVT = VP // 512
KC = D // 128  # 6 contraction chunks of 128
LN_EPS = 1e-12

_prog_cache = {}


def _build_program(with_dense_bias):
    import concourse.bacc as bacc
    import concourse.tile as tile
    from concourse import mybir
    from concourse.bass import ts
    from concourse.masks import make_identity

    fp32 = mybir.dt.float32
    fp32r = mybir.dt.float32r
    bf16 = mybir.dt.bfloat16
    AF = mybir.ActivationFunctionType
    ALU = mybir.AluOpType

    nc = bacc.Bacc("TRN2", target_bir_lowering=False, debug=False, num_devices=8)

    srcT = nc.dram_tensor("srcT", [H, S, Q], fp32, kind="ExternalInput")
    kT = nc.dram_tensor("kT", [128, KC, S], fp32r, kind="ExternalInput")
    vw = nc.dram_tensor("vw", [128, H, 4, 128], fp32r, kind="ExternalInput")
    wb = nc.dram_tensor("wb", [128, 4, 128], fp32r, kind="ExternalInput")
    pqT = nc.dram_tensor("pqT", [128, KC, Q], fp32r, kind="ExternalInput")
    dw = nc.dram_tensor("dw", [128, KC, D], fp32r, kind="ExternalInput")
    db = nc.dram_tensor("db", [1, D], fp32r, kind="ExternalInput")
    ones1d = nc.dram_tensor("ones1d", [1, 128], fp32r, kind="ExternalInput")
    decw = nc.dram_tensor("decw", [NVT, 128, KC, 512], bf16, kind="ExternalInput")
    out = nc.dram_tensor("out", [Q, VP], bf16, kind="ExternalOutput")

    with tile.TileContext(nc) as tc:
        with (
            tc.tile_pool(name="const", bufs=1) as const,
            tc.tile_pool(name="srcp", bufs=4) as srcp,
            tc.tile_pool(name="etp", bufs=2) as etp,
            tc.tile_pool(name="work", bufs=3) as work,
            tc.tile_pool(name="decin", bufs=3) as decin,
            tc.tile_pool(name="outp", bufs=4) as outp,
            tc.tile_pool(name="ps", bufs=8, space="PSUM") as ps,
        ):
            # resident tensors
            kT_sb = const.tile([128, KC, S], fp32r)
            nc.sync.dma_start(kT_sb[:], kT[:])
            pq_sb = const.tile([128, KC, Q], fp32r)
            nc.sync.dma_start(pq_sb[:], pqT[:])
            vw_sb = const.tile([128, H, 4, 128], fp32r)
            nc.sync.dma_start(vw_sb[:], vw[:])
            wb_sb = const.tile([128, 4, 128], fp32r)
            nc.sync.dma_start(wb_sb[:], wb[:])
            dw_sb = const.tile([128, KC, D], fp32r)
            nc.sync.dma_start(dw_sb[:], dw[:])
            db_sb = const.tile([1, D], fp32r)
            nc.sync.dma_start(db_sb[:], db[:])
            ident = const.tile([128, 128], bf16)
            make_identity(nc, ident[:])
            eps_sb = const.tile([128, 1], fp32)
            nc.vector.memset(eps_sb[:], LN_EPS)
            ssT_sb = const.tile([128, KC, Q], fp32r)
            hdnT_sb = const.tile([128, KC, Q], bf16)
            if with_dense_bias:
                ones1 = const.tile([1, 128], fp32r)
                nc.sync.dma_start(ones1[:], ones1d[:])

            # ---- attention, one head at a time ----
            for h in range(H):
                c0 = h // 2
                po = (h % 2) * 64
                et = etp.tile([128, 4, Q], fp32r, tag="et")
                for jc in range(4):
                    ps_sc = ps.tile([128, Q], fp32, tag="ps")
                    nc.tensor.matmul(
                        ps_sc[:],
                        lhsT=kT_sb[po : po + 64, c0, ts(jc, 128)],
                        rhs=pq_sb[po : po + 64, c0, :],
                        start=True,
                        stop=True,
                    )
                    src_t = srcp.tile([128, Q], fp32, tag="src")
                    nc.sync.dma_start(src_t[:], srcT[h, ts(jc, 128), :])
                    tmp = work.tile([128, Q], fp32, tag="tmp")
                    nc.vector.tensor_add(out=tmp[:], in0=ps_sc[:], in1=src_t[:])
                    nc.scalar.activation(out=et[:, jc, :], in_=tmp[:], func=AF.Exp)
                # alpha @ v (unnormalized, head cols at partitions po:po+64)
                ps_ss = ps.tile([128, Q], fp32, tag="ps")
                for jc in range(4):
                    nc.tensor.matmul(
                        ps_ss[:],
                        lhsT=vw_sb[:, h, jc, :],
                        rhs=et[:, jc, :],
                        start=(jc == 0),
                        stop=(jc == 3),
                    )
                # denom replicated across all 128 partitions
                ps_den = ps.tile([128, Q], fp32, tag="ps")
                for jc in range(4):
                    nc.tensor.matmul(
                        ps_den[:],
                        lhsT=wb_sb[:, jc, :],
                        rhs=et[:, jc, :],
                        start=(jc == 0),
                        stop=(jc == 3),
                    )
                rec = work.tile([128, Q], fp32, tag="rec")
                nc.vector.reciprocal(
                    out=rec[po : po + 64, :], in_=ps_den[po : po + 64, :]
                )
                nc.vector.tensor_mul(
                    out=ssT_sb[po : po + 64, c0, :],
                    in0=ps_ss[po : po + 64, :],
                    in1=rec[po : po + 64, :],
                )

            # ---- dense -> gelu -> layernorm -> transpose ----
            for it in range(4):
                ps_d0 = ps.tile([128, 512], fp32, tag="ps")
                ps_d1 = ps.tile([128, 256], fp32, tag="ps")
                for kc in range(KC):
                    lhsT = ssT_sb[:, kc, ts(it, 128)]
                    nc.tensor.matmul(
                        ps_d0[:],
                        lhsT=lhsT,
                        rhs=dw_sb[:, kc, 0:512],
                        start=(kc == 0),
                        stop=(kc == KC - 1) and not with_dense_bias,
                    )
                    nc.tensor.matmul(
                        ps_d1[:],
                        lhsT=lhsT,
                        rhs=dw_sb[:, kc, 512:768],
                        start=(kc == 0),
                        stop=(kc == KC - 1) and not with_dense_bias,
                    )
                if with_dense_bias:
                    nc.tensor.matmul(
                        ps_d0[:],
                        lhsT=ones1[:],
                        rhs=db_sb[0:1, 0:512],
                        start=False,
                        stop=True,
                    )
                    nc.tensor.matmul(
                        ps_d1[:],
                        lhsT=ones1[:],
                        rhs=db_sb[0:1, 512:768],
                        start=False,
                        stop=True,
                    )
                hdn = work.tile([128, D], fp32, tag="hdn")
                nc.scalar.activation(out=hdn[:, 0:512], in_=ps_d0[:], func=AF.Gelu)
                nc.scalar.activation(out=hdn[:, 512:768], in_=ps_d1[:], func=AF.Gelu)
                stats = work.tile([128, 3, 6], fp32, tag="stats")
                for sg in range(3):
                    nc.vector.bn_stats(
                        out=stats[:, sg, :], in_=hdn[:, sg * 256 : (sg + 1) * 256]
                    )
                mv = work.tile([128, 2], fp32, tag="mv")
                nc.vector.bn_aggr(out=mv[:], in_=stats[:])
                rs = work.tile([128, 1], fp32, tag="rs")
                nc.scalar.activation(
                    out=rs[:], in_=mv[:, 1:2], func=AF.Sqrt, bias=eps_sb[:]
                )
                nc.vector.reciprocal(out=rs[:], in_=rs[:])
                hdnn = work.tile([128, D], bf16, tag="hdnn")
                nc.vector.tensor_scalar(
                    out=hdnn[:],
                    in0=hdn[:],
                    scalar1=mv[:, 0:1],
                    scalar2=rs[:],
                    op0=ALU.subtract,
                    op1=ALU.mult,
                )
                for ec in range(KC):
                    ps_t = ps.tile([128, 128], bf16, tag="ps")
                    nc.tensor.transpose(ps_t[:], hdnn[:, ts(ec, 128)], ident[:])
                    nc.vector.tensor_copy(
                        out=hdnT_sb[:, ec, ts(it, 128)], in_=ps_t[:]
                    )

            # ---- decoder: logits[i, v] = hdnT.T @ decw ----
            for vt in range(NVT):
                dwt = decin.tile([128, KC, 512], bf16, tag="dwt")
                nc.sync.dma_start(dwt[:], decw[vt])
                for it in range(4):
                    ps_o = ps.tile([128, 512], fp32, tag="ps")
                    for kc in range(KC):
                        nc.tensor.matmul(
                            ps_o[:],
                            lhsT=hdnT_sb[:, kc, ts(it, 128)],
                            rhs=dwt[:, kc, :],
                            start=(kc == 0),
                            stop=(kc == KC - 1),
                        )
                    ob = outp.tile([128, 512], bf16, tag="ob")
                    if (vt * 4 + it) % 2 == 0:
                        nc.vector.tensor_copy(out=ob[:], in_=ps_o[:])
                    else:
                        nc.scalar.activation(out=ob[:], in_=ps_o[:], func=AF.Copy)
                    nc.sync.dma_start(out[ts(it, 128), ts(vt, 512)], ob[:])

    nc.compile()
    return nc


def _get_program(with_dense_bias):
    key = bool(with_dense_bias)
    if key not in _prog_cache:
        _prog_cache[key] = _build_program(key)
    return _prog_cache[key]


def _prep_inputs(inputs):
    f32 = np.float32
    src_scores = np.asarray(inputs["src_scores"], f32)
    k_full = np.asarray(inputs["seq_hidden_k"], f32)
    v_full = np.asarray(inputs["seq_hidden_v"], f32)
    mask = np.asarray(inputs["seq_mask"], f32)
    pos_emb = np.asarray(inputs["pos_emb"], f32)
    pos_proj_w = np.asarray(inputs["pos_proj_w"], f32)
    pos_proj_b = np.asarray(inputs["pos_proj_b"], f32)
    dense_w = np.asarray(inputs["dense_w"], f32)
    dense_b = np.asarray(inputs["dense_b"], f32)
    ln_g = np.asarray(inputs["ln_g"], f32)
    ln_b = np.asarray(inputs["ln_b"], f32)
    dec_w = np.asarray(inputs["dec_w"], f32)
    dec_b = np.asarray(inputs["dec_b"], f32)

    # shared (parameter) tensors
    pos_q = pos_emb[:Q] @ pos_proj_w + pos_proj_b  # [Q, D]
    pqT = np.ascontiguousarray(pos_q.reshape(Q, KC, 128).transpose(2, 1, 0))
    dw_host = np.ascontiguousarray(
        dense_w.reshape(KC, 128, D).transpose(1, 0, 2)
    )  # [128, KC, D]
    db_host = np.ascontiguousarray(dense_b.reshape(1, D))

    # fold LN gamma into decoder weight; LN beta + dec bias handled on host
    dec_wT = np.ascontiguousarray(dec_w.T) * ln_g[:, None]  # [D, V]
    dec_wT_p = np.zeros((D, VP), f32)
    dec_wT_p[:, :V] = dec_wT
    decw_host = np.ascontiguousarray(
        dec_wT_p.reshape(KC, 128, NVT, 512).transpose(2, 1, 0, 3)
    ).astype(ml_dtypes.bfloat16)  # [NVT, 128, KC, 512]
    dec_b_eff = dec_b + dec_w @ ln_b  # [V]

    w = 1.0 - mask  # [B, S]
    vp = v_full * w[:, :, None]  # masked v rows zeroed

    in_maps = []
    for b in range(B):
        srcT_b = np.ascontiguousarray(src_scores[b].transpose(0, 2, 1))
        kT_b = np.ascontiguousarray(
            k_full[b].reshape(S, KC, 128).transpose(2, 1, 0)
        )  # [128, KC, S]
        # vw: lhsT tiles [j-in-chunk, 128] per (head, jc); head parity picks
        # which 64-column half holds v so the output lands on partitions po:po+64
        vz = np.zeros((128, H, 4, 128), f32)
        t = vp[b].reshape(4, 128, H, HD).transpose(1, 2, 0, 3)  # [128, H, 4, HD]
        vz[:, 0::2, :, 0:64] = t[:, 0::2]
        vz[:, 1::2, :, 64:128] = t[:, 1::2]
        wb_b = np.ascontiguousarray(
            np.broadcast_to(w[b].reshape(4, 128).T[:, :, None], (128, 4, 128))
        )
        in_maps.append(
            dict(
                srcT=srcT_b,
                kT=kT_b,
                vw=vz,
                wb=wb_b,
                pqT=pqT,
                dw=dw_host,
                db=db_host,
                ones1d=np.ones((1, 128), f32),
                decw=decw_host,
            )
        )
    return in_maps, dec_b_eff, np.any(dense_b != 0.0)


def _run(inputs, trace=False):
    from concourse.bass_utils import run_bass_kernel_spmd

    in_maps, dec_b_eff, with_dense_bias = _prep_inputs(inputs)
    nc = _get_program(with_dense_bias)
    res = run_bass_kernel_spmd(nc, in_maps, list(range(B)), trace=trace)
    out_full = np.empty((B, Q, V), np.float32)
    for b in range(B):
        out_full[b] = res.results[b]["out"][:, :V].astype(np.float32)
    if np.any(dec_b_eff != 0.0):
        out_full += dec_b_eff[None, None, :]
    return out_full, res


def kernel(**inputs):
    out, _ = _run(inputs, trace=False)
    return out
